# revision 16
# baseline (speedup 1.0000x reference)
"""Causal multi-head self-attention (RoPE) Trainium2 Bass kernel.

Problem: x[4,2048,1024] f32, Wq/Wk/Wv/Wo[1024,1024], token_positions[2048].
  q,k,v = x@W.T per head (16 heads, dk=64); RoPE(q,k); causal softmax(q k^T/8) @ v;
  concat heads @ Wo.T.

Sharding (8 cores): core c -> batch b=c//2, head-group hg=c%2 (8 heads each).
Each core computes a partial output (its 8 heads' contribution through Wo);
host sums the two partials per batch.

On-chip layouts (per core):
  xT      [128, 8, 2048] f32r   : x[b].T chunked over d_model (DMA-streamed)
  qT/kT   [128, 512] bf16 per (pair, s-tile): rows = rope-permuted dims of a
          head pair: [A-even(0:32) A-odd(32:64) B-even(64:96) B-odd(96:128)]
  v       [128, 512] bf16 per s-chunk (8 heads x 64)
  scoresT [128, 2, 512] psum per chunk; exp on ACT -> attnT bf16; causal mask
          via gpsimd affine_select on diagonal blocks; AV pair col-tiled into
          one psum bank; denominator via ones[128,64] matmuls (broadcast over
          64 rows) into a second bank; one reciprocal_approx_fast + one
          tensor_mul normalizes; Wo projection from normalized outT.
"""

import os
from contextlib import ExitStack

import numpy as np
import ml_dtypes

import concourse.bass as bass
import concourse.tile as tile
from concourse import bacc, mybir
from concourse import bass_utils
from concourse._compat import with_exitstack

P = 128
B, S, D = 4, 2048, 1024
NHEAD, DK = 16, 64
HPC = 8      # heads per core
NPAIR = 4    # head pairs per core
DCH = 8      # d_model 128-chunks
NQT = 4      # q tiles of 512
SQT = 512
THETA = 10000.0
SCALE = 0.125          # 1/sqrt(dk)

F32 = mybir.dt.float32
F32R = mybir.dt.float32r
BF16 = mybir.dt.bfloat16

ROWSPLIT = os.environ.get("K_ROWSPLIT", "0") == "1"
PROJ16 = os.environ.get("K_PROJ16", "1") == "1"
PDT = BF16 if PROJ16 else F32R
PNP = ml_dtypes.bfloat16 if PROJ16 else np.float32
_STATE = None  # compile cache


@with_exitstack
def _attn_kernel(ctx: ExitStack, tc: tile.TileContext, out_ap, ins):
    nc = tc.nc
    xT, wq, wk, wv, wo, cosF, sinS, tri = ins

    wpool = ctx.enter_context(tc.tile_pool(name="w", bufs=1))
    xpool = ctx.enter_context(tc.tile_pool(name="x", bufs=2))
    qkpool = ctx.enter_context(tc.tile_pool(name="qk", bufs=1))
    vpool = ctx.enter_context(tc.tile_pool(name="v", bufs=1))
    rpool = ctx.enter_context(tc.tile_pool(name="rope", bufs=3))
    apool = ctx.enter_context(tc.tile_pool(name="attn", bufs=4))
    npool = ctx.enter_context(tc.tile_pool(name="norm", bufs=1))
    rcpool = ctx.enter_context(tc.tile_pool(name="rcp", bufs=2))
    wopool = ctx.enter_context(tc.tile_pool(name="wos", bufs=3))
    # PSUM (8 banks): psS 2x[128,2,512]=4, psO 1x{o0,o1}=2, psM 2x[128,512]=2
    psS = ctx.enter_context(tc.tile_pool(name="psS", bufs=2, space="PSUM"))
    psO = ctx.enter_context(tc.tile_pool(name="psO", bufs=1, space="PSUM"))
    psM = ctx.enter_context(tc.tile_pool(name="psM", bufs=2, space="PSUM"))

    # ---- resident constants ----
    wq_sb = wpool.tile([P, DCH, NPAIR, P], PDT, tag="wq")
    nc.sync.dma_start(wq_sb[:], wq)
    wk_sb = wpool.tile([P, DCH, NPAIR, P], PDT, tag="wk")
    nc.sync.dma_start(wk_sb[:], wk)
    wv_sb = wpool.tile([P, DCH, HPC * DK], PDT, tag="wv")
    nc.sync.dma_start(wv_sb[:], wv)
    wo_sb = wpool.tile([P, NPAIR, D], BF16, tag="wo")
    nc.sync.dma_start(wo_sb[:], wo)
    cos_sb = wpool.tile([P, S], BF16, tag="cos")
    nc.sync.dma_start(cos_sb[:], cosF)
    sin_sb = wpool.tile([P, S], BF16, tag="sin")
    nc.sync.dma_start(sin_sb[:], sinS)
    tri_sb = wpool.tile([P, P], BF16, tag="tri")
    nc.sync.dma_start(tri_sb[:], tri)

    qk_tiles = {}   # (proj, pair, stile) -> tile [128, 512] bf16
    v_tiles = {}    # schunk -> tile [128, 512] bf16
    nrm_tiles = {}  # (pair, qtile) -> tile [128, 512] bf16

    exp_fn = mybir.ActivationFunctionType.Exp

    def phase_a(t):
        xb = xpool.tile([P, DCH, SQT], PDT, tag="xb")
        nc.sync.dma_start(xb[:], xT[:, :, t * SQT:(t + 1) * SQT])
        s_sl = slice(t * SQT, (t + 1) * SQT)
        for p in range(NPAIR):
            for proj, w_sb in (("q", wq_sb), ("k", wk_sb)):
                ps = psM.tile([P, SQT], F32, tag="m")
                if ROWSPLIT:
                    for c in range(DCH):
                        for hf in range(2):
                            nc.tensor.matmul(
                                ps[:], w_sb[64 * hf:64 * hf + 64, c, p],
                                xb[64 * hf:64 * hf + 64, c],
                                start=(c == 0 and hf == 0),
                                stop=(c == DCH - 1 and hf == 1))
                else:
                    for c in range(DCH):
                        nc.tensor.matmul(ps[:], w_sb[:, c, p], xb[:, c],
                                         start=(c == 0), stop=(c == DCH - 1))
                # RoPE: one psum evacuation (releases the psum bank fast),
                # swap-copies on idle GPSIMD, bf16 2x multiplies/add on DVE.
                pb = rpool.tile([P, SQT], BF16, tag="pb")
                nc.vector.tensor_copy(pb[:], ps[:])
                sw = rpool.tile([P, SQT], BF16, tag="sw")
                for blk, src in ((0, 32), (1, 0), (2, 96), (3, 64)):
                    nc.gpsimd.tensor_copy(sw[32 * blk:32 * blk + 32],
                                          pb[src:src + 32])
                u = rpool.tile([P, SQT], BF16, tag="u")
                nc.vector.tensor_mul(u[:], pb[:], cos_sb[:, s_sl])
                w_ = rpool.tile([P, SQT], BF16, tag="wt")
                nc.vector.tensor_mul(w_[:], sw[:], sin_sb[:, s_sl])
                qt = qkpool.tile([P, SQT], BF16, tag=f"{proj}{p}_{t % 2 if proj == chr(113) else t}")
                nc.vector.tensor_add(qt[:], u[:], w_[:])
                qk_tiles[(proj, p, t)] = qt
        for sc4 in range(4):
            sc = 4 * t + sc4
            ps = psM.tile([P, SQT], F32, tag="m")
            if ROWSPLIT:
                for c in range(DCH):
                    for hf in range(2):
                        nc.tensor.matmul(
                            ps[:], xb[64 * hf:64 * hf + 64, c, 128 * sc4:128 * sc4 + 128],
                            wv_sb[64 * hf:64 * hf + 64, c],
                            start=(c == 0 and hf == 0),
                            stop=(c == DCH - 1 and hf == 1))
            else:
                for c in range(DCH):
                    nc.tensor.matmul(ps[:], xb[:, c, 128 * sc4:128 * sc4 + 128],
                                     wv_sb[:, c], start=(c == 0), stop=(c == DCH - 1))
            va = vpool.tile([P, HPC, 2 * DK], BF16, tag=f"v{sc}")
            nc.vector.tensor_copy(
                va[:, :, 0:DK], ps[:].rearrange("p (h d) -> p h d", d=DK))
            nc.vector.memset(va[:, :, DK:2 * DK], 1.0)
            v_tiles[sc] = va

    def phase_b(t):
        for p in range(NPAIR):
            qt = qk_tiles[("q", p, t)]
            oh = [psO.tile([P, SQT], F32, tag=f"o{h}", name=f"oh{h}")
                  for h in range(2)]
            nch = 4 * t + 4
            for kc in range(nch):
                delta = max(0, 128 * kc - SQT * t)
                kt = qk_tiles[("k", p, kc // 4)]
                ci = kc % 4
                sT = psS.tile([P, 2, SQT], F32, tag="s")
                for h in range(2):
                    nc.tensor.matmul(
                        sT[:, h, delta:], kt[64 * h:64 * h + 64, 128 * ci:128 * ci + 128],
                        qt[64 * h:64 * h + 64, delta:], start=True, stop=True)
                at = apool.tile([P, 2, SQT], BF16, tag="a")
                nc.scalar.activation(at[:, :, delta:], sT[:, :, delta:], exp_fn,
                                     scale=SCALE)
                if 128 * kc >= SQT * t:
                    # diagonal block: zero attn where q < k (gpsimd)
                    for h in range(2):
                        nc.gpsimd.affine_select(
                            out=at[:, h, delta:delta + 128],
                            in_=at[:, h, delta:delta + 128],
                            compare_op=mybir.AluOpType.is_ge,
                            fill=0.0, base=0,
                            pattern=[[1, 128]], channel_multiplier=-1)
                va = v_tiles[kc]
                st_, sp_ = (kc == 0), (kc == nch - 1)
                for h in range(2):
                    nc.tensor.matmul(
                        oh[h][:, delta:], va[:, 2 * p + h, :],
                        at[:, h, delta:], start=st_, stop=sp_)
            onrm = npool.tile([P, SQT], BF16, tag=f"n{p}_{t % 2}")
            dnb = rcpool.tile([P, SQT], F32, tag="dnb")
            for h in range(2):
                nc.vector.tensor_copy(dnb[64 * h:64 * h + 64], oh[h][64:128, :])
            rc = rcpool.tile([P, SQT], F32, tag="rc")
            nc.vector.reciprocal_approx_fast(rc[:], dnb[:])
            for h in range(2):
                nc.vector.tensor_mul(onrm[64 * h:64 * h + 64],
                                     oh[h][0:64, :], rc[64 * h:64 * h + 64])
            nrm_tiles[(p, t)] = onrm

    def phase_wo(t):
        for qs in range(4):
            for nh in range(2):
                wps = psM.tile([P, SQT], F32, tag="m")
                for p in range(NPAIR):
                    if ROWSPLIT:
                        for hf in range(2):
                            nc.tensor.matmul(
                                wps[:],
                                nrm_tiles[(p, t)][64 * hf:64 * hf + 64,
                                                  128 * qs:128 * qs + 128],
                                wo_sb[64 * hf:64 * hf + 64, p,
                                      SQT * nh:SQT * (nh + 1)],
                                start=(p == 0 and hf == 0),
                                stop=(p == NPAIR - 1 and hf == 1))
                    else:
                        nc.tensor.matmul(
                            wps[:], nrm_tiles[(p, t)][:, 128 * qs:128 * qs + 128],
                            wo_sb[:, p, SQT * nh:SQT * (nh + 1)],
                            start=(p == 0), stop=(p == NPAIR - 1))
                st = wopool.tile([P, SQT], F32, tag="wo")
                if nh == 0:
                    nc.vector.tensor_copy(st[:], wps[:])
                else:
                    nc.scalar.copy(st[:], wps[:])
                nc.sync.dma_start(
                    out_ap[SQT * t + 128 * qs:SQT * t + 128 * qs + 128,
                           SQT * nh:SQT * (nh + 1)], st[:])

    for t in range(NQT):
        phase_a(t)
        phase_b(t)
        phase_wo(t)


def _build():
    nc = bacc.Bacc("TRN2", target_bir_lowering=False, debug=False, num_devices=8)
    ins = [
        nc.dram_tensor("xT", [P, DCH, S], PDT, kind="ExternalInput").ap(),
        nc.dram_tensor("wq", [P, DCH, NPAIR, P], PDT, kind="ExternalInput").ap(),
        nc.dram_tensor("wk", [P, DCH, NPAIR, P], PDT, kind="ExternalInput").ap(),
        nc.dram_tensor("wv", [P, DCH, HPC * DK], PDT, kind="ExternalInput").ap(),
        nc.dram_tensor("wo", [P, NPAIR, D], BF16, kind="ExternalInput").ap(),
        nc.dram_tensor("cosF", [P, S], BF16, kind="ExternalInput").ap(),
        nc.dram_tensor("sinS", [P, S], BF16, kind="ExternalInput").ap(),
        nc.dram_tensor("tri", [P, P], BF16, kind="ExternalInput").ap(),
    ]
    out_ap = nc.dram_tensor("out", [S, D], F32, kind="ExternalOutput").ap()
    with tile.TileContext(nc) as tc:
        _attn_kernel(tc, out_ap, ins)
    nc.compile()
    return nc


def _host_prep(x, Wq, Wk, Wv, Wo, token_positions):
    """Build the 8 per-core input maps."""
    x = np.asarray(x, dtype=np.float32)
    Wq = np.asarray(Wq, dtype=np.float32)
    Wk = np.asarray(Wk, dtype=np.float32)
    Wv = np.asarray(Wv, dtype=np.float32)
    Wo = np.asarray(Wo, dtype=np.float32)
    pos = np.asarray(token_positions).astype(np.float64)

    # RoPE tables: rows 0:32 freq-major (even dims), repeated for the 4
    # 32-row blocks; sin signed [-,+,-,+] to implement the rotation swap.
    freqs = 1.0 / (THETA ** (np.arange(0, DK, 2, dtype=np.float64) / DK))  # [32]
    ang = pos[:, None] * freqs[None, :]          # [S, 32]
    cosT = np.cos(ang).T.astype(np.float32)      # [32, S]
    sinT = np.sin(ang).T.astype(np.float32)
    cosF = np.tile(cosT, (4, 1)).astype(ml_dtypes.bfloat16)
    sinS = np.concatenate([-sinT, sinT, -sinT, sinT], 0).astype(ml_dtypes.bfloat16)
    kk = np.arange(P)[:, None]
    qq = np.arange(P)[None, :]
    tri = np.where(qq >= kk, 0.0, -30000.0).astype(ml_dtypes.bfloat16)

    xTr = [np.ascontiguousarray(
        x[b].T.reshape(DCH, P, S).transpose(1, 0, 2)).astype(PNP) for b in range(B)]

    def wqk_arr(W, hg):
        perm = np.empty((NPAIR, P), np.int64)
        for p in range(NPAIR):
            hA, hB = 8 * hg + 2 * p, 8 * hg + 2 * p + 1
            perm[p] = np.concatenate([
                DK * hA + np.arange(0, DK, 2), DK * hA + np.arange(1, DK, 2),
                DK * hB + np.arange(0, DK, 2), DK * hB + np.arange(1, DK, 2)])
        a = W[perm]                                  # [4, 128, 1024]
        a = a.reshape(NPAIR, P, DCH, P).transpose(3, 2, 0, 1)  # [pi, c, p, m]
        return np.ascontiguousarray(a).astype(PNP)

    def wv_arr(hg):
        a = Wv[DK * HPC * hg: DK * HPC * (hg + 1)].T   # [1024, 512]
        return np.ascontiguousarray(
            a.reshape(DCH, P, HPC * DK).transpose(1, 0, 2)).astype(PNP)

    def wo_arr(hg):
        a = Wo[:, DK * HPC * hg: DK * HPC * (hg + 1)].T  # [512, 1024]
        return np.ascontiguousarray(
            a.reshape(NPAIR, P, D).transpose(1, 0, 2)).astype(ml_dtypes.bfloat16)

    in_maps = []
    for c in range(8):
        b, hg = c // 2, c % 2
        in_maps.append({
            "xT": xTr[b],
            "wq": wqk_arr(Wq, hg), "wk": wqk_arr(Wk, hg), "wv": wv_arr(hg),
            "wo": wo_arr(hg),
            "cosF": cosF, "sinS": sinS, "tri": tri,
        })
    return in_maps


def prepare(**inputs):
    """Returns (nc, in_maps). Exposed for test.py's traced runs."""
    global _STATE
    if _STATE is None:
        _STATE = _build()
    return _STATE, _host_prep(**inputs)


def kernel(**inputs):
    nc, in_maps = prepare(**inputs)
    res = bass_utils.run_bass_kernel_spmd(nc, in_maps, core_ids=list(range(8)))
    out = np.empty((B, S, D), np.float32)
    for b in range(B):
        out[b] = res.results[2 * b]["out"] + res.results[2 * b + 1]["out"]
    return out


# revision 17
# speedup vs baseline: 1.5049x; 1.5049x over previous
"""Causal multi-head self-attention (RoPE) Trainium2 Bass kernel.

Problem: x[4,2048,1024] f32, Wq/Wk/Wv/Wo[1024,1024], token_positions[2048].
  q,k,v = x@W.T per head (16 heads, dk=64); RoPE(q,k); causal softmax(q k^T/8) @ v;
  concat heads @ Wo.T.

Sharding (8 cores): core c -> batch b=c//2, head-group hg=c%2 (8 heads each).
Each core computes a partial output (its 8 heads' contribution through Wo);
host sums the two partials per batch.

On-chip layouts (per core):
  xT      [128, 8, 2048] f32r   : x[b].T chunked over d_model (DMA-streamed)
  qT/kT   [128, 512] bf16 per (pair, s-tile): rows = rope-permuted dims of a
          head pair: [A-even(0:32) A-odd(32:64) B-even(64:96) B-odd(96:128)]
  v       [128, 512] bf16 per s-chunk (8 heads x 64)
  scoresT [128, 2, 512] psum per chunk; exp on ACT -> attnT bf16; causal mask
          via gpsimd affine_select on diagonal blocks; AV pair col-tiled into
          one psum bank; denominator via ones[128,64] matmuls (broadcast over
          64 rows) into a second bank; one reciprocal_approx_fast + one
          tensor_mul normalizes; Wo projection from normalized outT.
"""

import os
from contextlib import ExitStack

import numpy as np
import ml_dtypes

import concourse.bass as bass
import concourse.tile as tile
from concourse import bacc, mybir
from concourse import bass_utils
from concourse._compat import with_exitstack

P = 128
B, S, D = 4, 2048, 1024
NHEAD, DK = 16, 64
HPC = 8      # heads per core
NPAIR = 4    # head pairs per core
DCH = 8      # d_model 128-chunks
NQT = 4      # q tiles of 512
SQT = 512
THETA = 10000.0
SCALE = 0.125          # 1/sqrt(dk)

F32 = mybir.dt.float32
F32R = mybir.dt.float32r
BF16 = mybir.dt.bfloat16

ROWSPLIT = os.environ.get("K_ROWSPLIT", "0") == "1"
PROJ16 = os.environ.get("K_PROJ16", "1") == "1"
PDT = BF16 if PROJ16 else F32R
PNP = ml_dtypes.bfloat16 if PROJ16 else np.float32
_STATE = None  # compile cache


@with_exitstack
def _attn_kernel(ctx: ExitStack, tc: tile.TileContext, out_ap, ins):
    nc = tc.nc
    xT, wq, wk, wv, wo, cosF, sinS, tri = ins

    wpool = ctx.enter_context(tc.tile_pool(name="w", bufs=1))
    xpool = ctx.enter_context(tc.tile_pool(name="x", bufs=2))
    qkpool = ctx.enter_context(tc.tile_pool(name="qk", bufs=1))
    vpool = ctx.enter_context(tc.tile_pool(name="v", bufs=1))
    rpool = ctx.enter_context(tc.tile_pool(name="rope", bufs=4))
    apool = ctx.enter_context(tc.tile_pool(name="attn", bufs=4))
    npool = ctx.enter_context(tc.tile_pool(name="norm", bufs=1))
    rcpool = ctx.enter_context(tc.tile_pool(name="rcp", bufs=2))
    wopool = ctx.enter_context(tc.tile_pool(name="wos", bufs=3))
    # PSUM (8 banks): psS 2x[128,2,512]=4, psO 1x{o0,o1}=2, psM 2x[128,512]=2
    psS = ctx.enter_context(tc.tile_pool(name="psS", bufs=2, space="PSUM"))
    psO = ctx.enter_context(tc.tile_pool(name="psO", bufs=1, space="PSUM"))
    psM = ctx.enter_context(tc.tile_pool(name="psM", bufs=2, space="PSUM"))

    # ---- resident constants ----
    wq_sb = wpool.tile([P, DCH, NPAIR, P], PDT, tag="wq")
    nc.sync.dma_start(wq_sb[:], wq)
    wk_sb = wpool.tile([P, DCH, NPAIR, P], PDT, tag="wk")
    nc.sync.dma_start(wk_sb[:], wk)
    wv_sb = wpool.tile([P, DCH, HPC * DK], PDT, tag="wv")
    nc.sync.dma_start(wv_sb[:], wv)
    wo_sb = wpool.tile([P, NPAIR, D], BF16, tag="wo")
    nc.sync.dma_start(wo_sb[:], wo)
    cos_sb = wpool.tile([P, S], BF16, tag="cos")
    nc.sync.dma_start(cos_sb[:], cosF)
    sin_sb = wpool.tile([P, S], BF16, tag="sin")
    nc.sync.dma_start(sin_sb[:], sinS)
    tri_sb = wpool.tile([P, P], BF16, tag="tri")
    nc.sync.dma_start(tri_sb[:], tri)

    qk_tiles = {}   # (proj, pair, stile) -> tile [128, 512] bf16
    v_tiles = {}    # schunk -> tile [128, 512] bf16
    nrm_tiles = {}  # (pair, qtile) -> tile [128, 512] bf16

    exp_fn = mybir.ActivationFunctionType.Exp

    def phase_a(t):
        xb = xpool.tile([P, DCH, SQT], PDT, tag="xb")
        nc.sync.dma_start(xb[:], xT[:, :, t * SQT:(t + 1) * SQT])
        s_sl = slice(t * SQT, (t + 1) * SQT)
        for p in range(NPAIR):
            for proj, w_sb in (("q", wq_sb), ("k", wk_sb)):
                ps = psM.tile([P, SQT], F32, tag="m")
                if ROWSPLIT:
                    for c in range(DCH):
                        for hf in range(2):
                            nc.tensor.matmul(
                                ps[:], w_sb[64 * hf:64 * hf + 64, c, p],
                                xb[64 * hf:64 * hf + 64, c],
                                start=(c == 0 and hf == 0),
                                stop=(c == DCH - 1 and hf == 1))
                else:
                    for c in range(DCH):
                        nc.tensor.matmul(ps[:], w_sb[:, c, p], xb[:, c],
                                         start=(c == 0), stop=(c == DCH - 1))
                # RoPE: one psum evacuation (releases the psum bank fast),
                # swap-copies on idle GPSIMD, bf16 2x multiplies/add on DVE.
                pb = rpool.tile([P, SQT], BF16, tag="pb")
                nc.vector.tensor_copy(pb[:], ps[:])
                sw = rpool.tile([P, SQT], BF16, tag="sw")
                for blk, src in ((0, 32), (1, 0), (2, 96), (3, 64)):
                    nc.sync.dma_start(sw[32 * blk:32 * blk + 32],
                                      pb[src:src + 32])
                u = rpool.tile([P, SQT], BF16, tag="u")
                nc.vector.tensor_mul(u[:], pb[:], cos_sb[:, s_sl])
                w_ = rpool.tile([P, SQT], BF16, tag="wt")
                nc.vector.tensor_mul(w_[:], sw[:], sin_sb[:, s_sl])
                qt = qkpool.tile([P, SQT], BF16, tag=f"{proj}{p}_{t % 2 if proj == chr(113) else t}")
                nc.vector.tensor_add(qt[:], u[:], w_[:])
                qk_tiles[(proj, p, t)] = qt
        for sc4 in range(4):
            sc = 4 * t + sc4
            ps = psM.tile([P, SQT], F32, tag="m")
            if ROWSPLIT:
                for c in range(DCH):
                    for hf in range(2):
                        nc.tensor.matmul(
                            ps[:], xb[64 * hf:64 * hf + 64, c, 128 * sc4:128 * sc4 + 128],
                            wv_sb[64 * hf:64 * hf + 64, c],
                            start=(c == 0 and hf == 0),
                            stop=(c == DCH - 1 and hf == 1))
            else:
                for c in range(DCH):
                    nc.tensor.matmul(ps[:], xb[:, c, 128 * sc4:128 * sc4 + 128],
                                     wv_sb[:, c], start=(c == 0), stop=(c == DCH - 1))
            va = vpool.tile([P, HPC, 2 * DK], BF16, tag=f"v{sc}")
            nc.vector.tensor_copy(
                va[:, :, 0:DK], ps[:].rearrange("p (h d) -> p h d", d=DK))
            nc.vector.memset(va[:, :, DK:2 * DK], 1.0)
            v_tiles[sc] = va

    def phase_b(t):
        for p in range(NPAIR):
            qt = qk_tiles[("q", p, t)]
            oh = [psO.tile([P, SQT], F32, tag=f"o{h}", name=f"oh{h}")
                  for h in range(2)]
            nch = 4 * t + 4
            for kc in range(nch):
                delta = max(0, 128 * kc - SQT * t)
                kt = qk_tiles[("k", p, kc // 4)]
                ci = kc % 4
                sT = psS.tile([P, 2, SQT], F32, tag="s")
                for h in range(2):
                    nc.tensor.matmul(
                        sT[:, h, delta:], kt[64 * h:64 * h + 64, 128 * ci:128 * ci + 128],
                        qt[64 * h:64 * h + 64, delta:], start=True, stop=True)
                at = apool.tile([P, 2, SQT], BF16, tag="a")
                nc.scalar.activation(at[:, :, delta:], sT[:, :, delta:], exp_fn,
                                     scale=SCALE)
                if 128 * kc >= SQT * t:
                    # diagonal block: zero attn where q < k (gpsimd)
                    for h in range(2):
                        nc.gpsimd.affine_select(
                            out=at[:, h, delta:delta + 128],
                            in_=at[:, h, delta:delta + 128],
                            compare_op=mybir.AluOpType.is_ge,
                            fill=0.0, base=0,
                            pattern=[[1, 128]], channel_multiplier=-1)
                va = v_tiles[kc]
                st_, sp_ = (kc == 0), (kc == nch - 1)
                for h in range(2):
                    nc.tensor.matmul(
                        oh[h][:, delta:], va[:, 2 * p + h, :],
                        at[:, h, delta:], start=st_, stop=sp_)
            onrm = npool.tile([P, SQT], BF16, tag=f"n{p}_{t % 2}")
            dnb = rcpool.tile([P, SQT], F32, tag="dnb")
            for h in range(2):
                nc.vector.tensor_copy(dnb[64 * h:64 * h + 64], oh[h][64:128, :])
            rc = rcpool.tile([P, SQT], F32, tag="rc")
            nc.vector.reciprocal_approx_fast(rc[:], dnb[:])
            for h in range(2):
                nc.vector.tensor_mul(onrm[64 * h:64 * h + 64],
                                     oh[h][0:64, :], rc[64 * h:64 * h + 64])
            nrm_tiles[(p, t)] = onrm

    def phase_wo(t):
        for qs in range(4):
            for nh in range(2):
                wps = psM.tile([P, SQT], F32, tag="m")
                for p in range(NPAIR):
                    if ROWSPLIT:
                        for hf in range(2):
                            nc.tensor.matmul(
                                wps[:],
                                nrm_tiles[(p, t)][64 * hf:64 * hf + 64,
                                                  128 * qs:128 * qs + 128],
                                wo_sb[64 * hf:64 * hf + 64, p,
                                      SQT * nh:SQT * (nh + 1)],
                                start=(p == 0 and hf == 0),
                                stop=(p == NPAIR - 1 and hf == 1))
                    else:
                        nc.tensor.matmul(
                            wps[:], nrm_tiles[(p, t)][:, 128 * qs:128 * qs + 128],
                            wo_sb[:, p, SQT * nh:SQT * (nh + 1)],
                            start=(p == 0), stop=(p == NPAIR - 1))
                st = wopool.tile([P, SQT], F32, tag="wo")
                if nh == 0:
                    nc.vector.tensor_copy(st[:], wps[:])
                else:
                    nc.scalar.copy(st[:], wps[:])
                nc.sync.dma_start(
                    out_ap[SQT * t + 128 * qs:SQT * t + 128 * qs + 128,
                           SQT * nh:SQT * (nh + 1)], st[:])

    for t in range(NQT):
        phase_a(t)
        phase_b(t)
        phase_wo(t)


def _build():
    nc = bacc.Bacc("TRN2", target_bir_lowering=False, debug=False, num_devices=8)
    ins = [
        nc.dram_tensor("xT", [P, DCH, S], PDT, kind="ExternalInput").ap(),
        nc.dram_tensor("wq", [P, DCH, NPAIR, P], PDT, kind="ExternalInput").ap(),
        nc.dram_tensor("wk", [P, DCH, NPAIR, P], PDT, kind="ExternalInput").ap(),
        nc.dram_tensor("wv", [P, DCH, HPC * DK], PDT, kind="ExternalInput").ap(),
        nc.dram_tensor("wo", [P, NPAIR, D], BF16, kind="ExternalInput").ap(),
        nc.dram_tensor("cosF", [P, S], BF16, kind="ExternalInput").ap(),
        nc.dram_tensor("sinS", [P, S], BF16, kind="ExternalInput").ap(),
        nc.dram_tensor("tri", [P, P], BF16, kind="ExternalInput").ap(),
    ]
    out_ap = nc.dram_tensor("out", [S, D], F32, kind="ExternalOutput").ap()
    with tile.TileContext(nc) as tc:
        _attn_kernel(tc, out_ap, ins)
    nc.compile()
    return nc


def _host_prep(x, Wq, Wk, Wv, Wo, token_positions):
    """Build the 8 per-core input maps."""
    x = np.asarray(x, dtype=np.float32)
    Wq = np.asarray(Wq, dtype=np.float32)
    Wk = np.asarray(Wk, dtype=np.float32)
    Wv = np.asarray(Wv, dtype=np.float32)
    Wo = np.asarray(Wo, dtype=np.float32)
    pos = np.asarray(token_positions).astype(np.float64)

    # RoPE tables: rows 0:32 freq-major (even dims), repeated for the 4
    # 32-row blocks; sin signed [-,+,-,+] to implement the rotation swap.
    freqs = 1.0 / (THETA ** (np.arange(0, DK, 2, dtype=np.float64) / DK))  # [32]
    ang = pos[:, None] * freqs[None, :]          # [S, 32]
    cosT = np.cos(ang).T.astype(np.float32)      # [32, S]
    sinT = np.sin(ang).T.astype(np.float32)
    cosF = np.tile(cosT, (4, 1)).astype(ml_dtypes.bfloat16)
    sinS = np.concatenate([-sinT, sinT, -sinT, sinT], 0).astype(ml_dtypes.bfloat16)
    kk = np.arange(P)[:, None]
    qq = np.arange(P)[None, :]
    tri = np.where(qq >= kk, 0.0, -30000.0).astype(ml_dtypes.bfloat16)

    xTr = [np.ascontiguousarray(
        x[b].T.reshape(DCH, P, S).transpose(1, 0, 2)).astype(PNP) for b in range(B)]

    def wqk_arr(W, hg):
        perm = np.empty((NPAIR, P), np.int64)
        for p in range(NPAIR):
            hA, hB = 8 * hg + 2 * p, 8 * hg + 2 * p + 1
            perm[p] = np.concatenate([
                DK * hA + np.arange(0, DK, 2), DK * hA + np.arange(1, DK, 2),
                DK * hB + np.arange(0, DK, 2), DK * hB + np.arange(1, DK, 2)])
        a = W[perm]                                  # [4, 128, 1024]
        a = a.reshape(NPAIR, P, DCH, P).transpose(3, 2, 0, 1)  # [pi, c, p, m]
        return np.ascontiguousarray(a).astype(PNP)

    def wv_arr(hg):
        a = Wv[DK * HPC * hg: DK * HPC * (hg + 1)].T   # [1024, 512]
        return np.ascontiguousarray(
            a.reshape(DCH, P, HPC * DK).transpose(1, 0, 2)).astype(PNP)

    def wo_arr(hg):
        a = Wo[:, DK * HPC * hg: DK * HPC * (hg + 1)].T  # [512, 1024]
        return np.ascontiguousarray(
            a.reshape(NPAIR, P, D).transpose(1, 0, 2)).astype(ml_dtypes.bfloat16)

    in_maps = []
    for c in range(8):
        b, hg = c // 2, c % 2
        in_maps.append({
            "xT": xTr[b],
            "wq": wqk_arr(Wq, hg), "wk": wqk_arr(Wk, hg), "wv": wv_arr(hg),
            "wo": wo_arr(hg),
            "cosF": cosF, "sinS": sinS, "tri": tri,
        })
    return in_maps


def prepare(**inputs):
    """Returns (nc, in_maps). Exposed for test.py's traced runs."""
    global _STATE
    if _STATE is None:
        _STATE = _build()
    return _STATE, _host_prep(**inputs)


def kernel(**inputs):
    nc, in_maps = prepare(**inputs)
    res = bass_utils.run_bass_kernel_spmd(nc, in_maps, core_ids=list(range(8)))
    out = np.empty((B, S, D), np.float32)
    for b in range(B):
        out[b] = res.results[2 * b]["out"] + res.results[2 * b + 1]["out"]
    return out


# revision 19
# speedup vs baseline: 1.5826x; 1.0516x over previous
"""Causal multi-head self-attention (RoPE) Trainium2 Bass kernel.

Problem: x[4,2048,1024] f32, Wq/Wk/Wv/Wo[1024,1024], token_positions[2048].
  q,k,v = x@W.T per head (16 heads, dk=64); RoPE(q,k); causal softmax(q k^T/8) @ v;
  concat heads @ Wo.T.

Sharding (8 cores): core c -> batch b=c//2, head-group hg=c%2 (8 heads each).
Each core computes a partial output (its 8 heads' contribution through Wo);
host sums the two partials per batch.

On-chip layouts (per core):
  xT      [128, 8, 2048] f32r   : x[b].T chunked over d_model (DMA-streamed)
  qT/kT   [128, 512] bf16 per (pair, s-tile): rows = rope-permuted dims of a
          head pair: [A-even(0:32) A-odd(32:64) B-even(64:96) B-odd(96:128)]
  v       [128, 512] bf16 per s-chunk (8 heads x 64)
  scoresT [128, 2, 512] psum per chunk; exp on ACT -> attnT bf16; causal mask
          via gpsimd affine_select on diagonal blocks; AV pair col-tiled into
          one psum bank; denominator via ones[128,64] matmuls (broadcast over
          64 rows) into a second bank; one reciprocal_approx_fast + one
          tensor_mul normalizes; Wo projection from normalized outT.
"""

import os
from contextlib import ExitStack

import numpy as np
import ml_dtypes

import concourse.bass as bass
import concourse.tile as tile
from concourse import bacc, mybir
from concourse import bass_utils
from concourse._compat import with_exitstack

P = 128
B, S, D = 4, 2048, 1024
NHEAD, DK = 16, 64
HPC = 8      # heads per core
NPAIR = 4    # head pairs per core
DCH = 8      # d_model 128-chunks
NQT = 4      # q tiles of 512
SQT = 512
THETA = 10000.0
SCALE = 0.125          # 1/sqrt(dk)

F32 = mybir.dt.float32
F32R = mybir.dt.float32r
BF16 = mybir.dt.bfloat16

ROWSPLIT = os.environ.get("K_ROWSPLIT", "0") == "1"
PROJ16 = os.environ.get("K_PROJ16", "1") == "1"
PDT = BF16 if PROJ16 else F32R
PNP = ml_dtypes.bfloat16 if PROJ16 else np.float32
_STATE = None  # compile cache


@with_exitstack
def _attn_kernel(ctx: ExitStack, tc: tile.TileContext, out_ap, ins):
    nc = tc.nc
    xT, wq, wk, wv, wo, cosF, sinS, tri = ins

    wpool = ctx.enter_context(tc.tile_pool(name="w", bufs=1))
    xpool = ctx.enter_context(tc.tile_pool(name="x", bufs=2))
    qkpool = ctx.enter_context(tc.tile_pool(name="qk", bufs=1))
    vpool = ctx.enter_context(tc.tile_pool(name="v", bufs=1))
    rpool = ctx.enter_context(tc.tile_pool(name="rope", bufs=4))
    apool = ctx.enter_context(tc.tile_pool(name="attn", bufs=4))
    npool = ctx.enter_context(tc.tile_pool(name="norm", bufs=1))
    rcpool = ctx.enter_context(tc.tile_pool(name="rcp", bufs=2))
    wopool = ctx.enter_context(tc.tile_pool(name="wos", bufs=3))
    # PSUM (8 banks): psS 2x[128,2,512]=4, psO 1x{o0,o1}=2, psM 2x[128,512]=2
    psS = ctx.enter_context(tc.tile_pool(name="psS", bufs=2, space="PSUM"))
    psO = ctx.enter_context(tc.tile_pool(name="psO", bufs=1, space="PSUM"))
    psM = ctx.enter_context(tc.tile_pool(name="psM", bufs=2, space="PSUM"))

    # ---- resident constants ----
    wq_sb = wpool.tile([P, DCH, NPAIR, P], PDT, tag="wq")
    nc.sync.dma_start(wq_sb[:], wq)
    wk_sb = wpool.tile([P, DCH, NPAIR, P], PDT, tag="wk")
    nc.sync.dma_start(wk_sb[:], wk)
    wv_sb = wpool.tile([P, DCH, HPC * DK], PDT, tag="wv")
    nc.sync.dma_start(wv_sb[:], wv)
    wo_sb = wpool.tile([P, NPAIR, D], BF16, tag="wo")
    nc.sync.dma_start(wo_sb[:], wo)
    cos_sb = wpool.tile([P, S], BF16, tag="cos")
    nc.sync.dma_start(cos_sb[:], cosF)
    sin_sb = wpool.tile([P, S], BF16, tag="sin")
    nc.sync.dma_start(sin_sb[:], sinS)
    tri_sb = wpool.tile([P, P], BF16, tag="tri")
    nc.sync.dma_start(tri_sb[:], tri)

    qk_tiles = {}   # (proj, pair, stile) -> tile [128, 512] bf16
    v_tiles = {}    # schunk -> tile [128, 512] bf16
    nrm_tiles = {}  # (pair, qtile) -> tile [128, 512] bf16

    exp_fn = mybir.ActivationFunctionType.Exp

    def phase_a(t):
        xb = xpool.tile([P, DCH, SQT], PDT, tag="xb")
        nc.sync.dma_start(xb[:], xT[:, :, t * SQT:(t + 1) * SQT])
        s_sl = slice(t * SQT, (t + 1) * SQT)
        for p in range(NPAIR):
            for proj, w_sb in (("q", wq_sb), ("k", wk_sb)):
                ps = psM.tile([P, SQT], F32, tag="m")
                if ROWSPLIT:
                    for c in range(DCH):
                        for hf in range(2):
                            nc.tensor.matmul(
                                ps[:], w_sb[64 * hf:64 * hf + 64, c, p],
                                xb[64 * hf:64 * hf + 64, c],
                                start=(c == 0 and hf == 0),
                                stop=(c == DCH - 1 and hf == 1))
                else:
                    for c in range(DCH):
                        nc.tensor.matmul(ps[:], w_sb[:, c, p], xb[:, c],
                                         start=(c == 0), stop=(c == DCH - 1))
                # RoPE: one psum evacuation (releases the psum bank fast),
                # swap-copies on idle GPSIMD, bf16 2x multiplies/add on DVE.
                pb = rpool.tile([P, SQT], BF16, tag="pb")
                nc.vector.tensor_copy(pb[:], ps[:])
                sw = rpool.tile([P, SQT], BF16, tag="sw")
                for blk, src in ((0, 32), (1, 0), (2, 96), (3, 64)):
                    nc.sync.dma_start(sw[32 * blk:32 * blk + 32],
                                      pb[src:src + 32])
                u = rpool.tile([P, SQT], BF16, tag="u")
                nc.vector.tensor_mul(u[:], pb[:], cos_sb[:, s_sl])
                w_ = rpool.tile([P, SQT], BF16, tag="wt")
                nc.vector.tensor_mul(w_[:], sw[:], sin_sb[:, s_sl])
                qt = qkpool.tile([P, SQT], BF16, tag=f"{proj}{p}_{t % 2 if proj == chr(113) else t}")
                nc.vector.tensor_add(qt[:], u[:], w_[:])
                qk_tiles[(proj, p, t)] = qt
        for sc4 in range(4):
            sc = 4 * t + sc4
            ps = psM.tile([P, SQT], F32, tag="m")
            if ROWSPLIT:
                for c in range(DCH):
                    for hf in range(2):
                        nc.tensor.matmul(
                            ps[:], xb[64 * hf:64 * hf + 64, c, 128 * sc4:128 * sc4 + 128],
                            wv_sb[64 * hf:64 * hf + 64, c],
                            start=(c == 0 and hf == 0),
                            stop=(c == DCH - 1 and hf == 1))
            else:
                for c in range(DCH):
                    nc.tensor.matmul(ps[:], xb[:, c, 128 * sc4:128 * sc4 + 128],
                                     wv_sb[:, c], start=(c == 0), stop=(c == DCH - 1))
            va = vpool.tile([P, HPC, 2 * DK], BF16, tag=f"v{sc}")
            nc.vector.tensor_copy(
                va[:, :, 0:DK], ps[:].rearrange("p (h d) -> p h d", d=DK))
            nc.vector.memset(va[:, :, DK:2 * DK], 1.0)
            v_tiles[sc] = va

    def phase_b(t):
        for p in range(NPAIR):
            qt = qk_tiles[("q", p, t)]
            oh = [psO.tile([P, SQT], F32, tag=f"o{h}", name=f"oh{h}")
                  for h in range(2)]
            nch = 4 * t + 4
            for kc in range(nch):
                delta = max(0, 128 * kc - SQT * t)
                kt = qk_tiles[("k", p, kc // 4)]
                ci = kc % 4
                sT = psS.tile([P, 2, SQT], F32, tag="s")
                for h in range(2):
                    nc.tensor.matmul(
                        sT[:, h, delta:], kt[64 * h:64 * h + 64, 128 * ci:128 * ci + 128],
                        qt[64 * h:64 * h + 64, delta:], start=True, stop=True)
                at = apool.tile([P, 2, SQT], BF16, tag="a")
                nc.scalar.activation(at[:, :, delta:], sT[:, :, delta:], exp_fn,
                                     scale=SCALE)
                if 128 * kc >= SQT * t:
                    # diagonal block: zero attn where q < k (gpsimd)
                    for h in range(2):
                        nc.gpsimd.affine_select(
                            out=at[:, h, delta:delta + 128],
                            in_=at[:, h, delta:delta + 128],
                            compare_op=mybir.AluOpType.is_ge,
                            fill=0.0, base=0,
                            pattern=[[1, 128]], channel_multiplier=-1)
                va = v_tiles[kc]
                st_, sp_ = (kc == 0), (kc == nch - 1)
                for h in range(2):
                    nc.tensor.matmul(
                        oh[h][:, delta:], va[:, 2 * p + h, :],
                        at[:, h, delta:], start=st_, stop=sp_)
            onrm = npool.tile([P, SQT], BF16, tag=f"n{p}_{t % 2}")
            ohb = rcpool.tile([P, SQT], F32, tag="ohb")
            dnb = rcpool.tile([P, SQT], F32, tag="dnb")
            for h in range(2):
                # evacuate outT+denom; psum slot released after these copies
                nc.vector.tensor_copy(ohb[64 * h:64 * h + 64], oh[h][0:64, :])
                nc.vector.tensor_copy(dnb[64 * h:64 * h + 64], oh[h][64:128, :])
            rc = rcpool.tile([P, SQT], F32, tag="rc")
            nc.vector.reciprocal_approx_fast(rc[:], dnb[:])
            nc.vector.tensor_mul(onrm[:], ohb[:], rc[:])
            nrm_tiles[(p, t)] = onrm

    def phase_wo(t):
        for qs in range(4):
            for nh in range(2):
                wps = psM.tile([P, SQT], F32, tag="m")
                for p in range(NPAIR):
                    if ROWSPLIT:
                        for hf in range(2):
                            nc.tensor.matmul(
                                wps[:],
                                nrm_tiles[(p, t)][64 * hf:64 * hf + 64,
                                                  128 * qs:128 * qs + 128],
                                wo_sb[64 * hf:64 * hf + 64, p,
                                      SQT * nh:SQT * (nh + 1)],
                                start=(p == 0 and hf == 0),
                                stop=(p == NPAIR - 1 and hf == 1))
                    else:
                        nc.tensor.matmul(
                            wps[:], nrm_tiles[(p, t)][:, 128 * qs:128 * qs + 128],
                            wo_sb[:, p, SQT * nh:SQT * (nh + 1)],
                            start=(p == 0), stop=(p == NPAIR - 1))
                st = wopool.tile([P, SQT], F32, tag="wo")
                if nh == 0:
                    nc.vector.tensor_copy(st[:], wps[:])
                else:
                    nc.scalar.copy(st[:], wps[:])
                nc.sync.dma_start(
                    out_ap[SQT * t + 128 * qs:SQT * t + 128 * qs + 128,
                           SQT * nh:SQT * (nh + 1)], st[:])

    for t in range(NQT):
        phase_a(t)
        phase_b(t)
        phase_wo(t)


def _build():
    nc = bacc.Bacc("TRN2", target_bir_lowering=False, debug=False, num_devices=8)
    ins = [
        nc.dram_tensor("xT", [P, DCH, S], PDT, kind="ExternalInput").ap(),
        nc.dram_tensor("wq", [P, DCH, NPAIR, P], PDT, kind="ExternalInput").ap(),
        nc.dram_tensor("wk", [P, DCH, NPAIR, P], PDT, kind="ExternalInput").ap(),
        nc.dram_tensor("wv", [P, DCH, HPC * DK], PDT, kind="ExternalInput").ap(),
        nc.dram_tensor("wo", [P, NPAIR, D], BF16, kind="ExternalInput").ap(),
        nc.dram_tensor("cosF", [P, S], BF16, kind="ExternalInput").ap(),
        nc.dram_tensor("sinS", [P, S], BF16, kind="ExternalInput").ap(),
        nc.dram_tensor("tri", [P, P], BF16, kind="ExternalInput").ap(),
    ]
    out_ap = nc.dram_tensor("out", [S, D], F32, kind="ExternalOutput").ap()
    with tile.TileContext(nc) as tc:
        _attn_kernel(tc, out_ap, ins)
    nc.compile()
    return nc


def _host_prep(x, Wq, Wk, Wv, Wo, token_positions):
    """Build the 8 per-core input maps."""
    x = np.asarray(x, dtype=np.float32)
    Wq = np.asarray(Wq, dtype=np.float32)
    Wk = np.asarray(Wk, dtype=np.float32)
    Wv = np.asarray(Wv, dtype=np.float32)
    Wo = np.asarray(Wo, dtype=np.float32)
    pos = np.asarray(token_positions).astype(np.float64)

    # RoPE tables: rows 0:32 freq-major (even dims), repeated for the 4
    # 32-row blocks; sin signed [-,+,-,+] to implement the rotation swap.
    freqs = 1.0 / (THETA ** (np.arange(0, DK, 2, dtype=np.float64) / DK))  # [32]
    ang = pos[:, None] * freqs[None, :]          # [S, 32]
    cosT = np.cos(ang).T.astype(np.float32)      # [32, S]
    sinT = np.sin(ang).T.astype(np.float32)
    cosF = np.tile(cosT, (4, 1)).astype(ml_dtypes.bfloat16)
    sinS = np.concatenate([-sinT, sinT, -sinT, sinT], 0).astype(ml_dtypes.bfloat16)
    kk = np.arange(P)[:, None]
    qq = np.arange(P)[None, :]
    tri = np.where(qq >= kk, 0.0, -30000.0).astype(ml_dtypes.bfloat16)

    xTr = [np.ascontiguousarray(
        x[b].T.reshape(DCH, P, S).transpose(1, 0, 2)).astype(PNP) for b in range(B)]

    def wqk_arr(W, hg):
        perm = np.empty((NPAIR, P), np.int64)
        for p in range(NPAIR):
            hA, hB = 8 * hg + 2 * p, 8 * hg + 2 * p + 1
            perm[p] = np.concatenate([
                DK * hA + np.arange(0, DK, 2), DK * hA + np.arange(1, DK, 2),
                DK * hB + np.arange(0, DK, 2), DK * hB + np.arange(1, DK, 2)])
        a = W[perm]                                  # [4, 128, 1024]
        a = a.reshape(NPAIR, P, DCH, P).transpose(3, 2, 0, 1)  # [pi, c, p, m]
        return np.ascontiguousarray(a).astype(PNP)

    def wv_arr(hg):
        a = Wv[DK * HPC * hg: DK * HPC * (hg + 1)].T   # [1024, 512]
        return np.ascontiguousarray(
            a.reshape(DCH, P, HPC * DK).transpose(1, 0, 2)).astype(PNP)

    def wo_arr(hg):
        a = Wo[:, DK * HPC * hg: DK * HPC * (hg + 1)].T  # [512, 1024]
        return np.ascontiguousarray(
            a.reshape(NPAIR, P, D).transpose(1, 0, 2)).astype(ml_dtypes.bfloat16)

    in_maps = []
    for c in range(8):
        b, hg = c // 2, c % 2
        in_maps.append({
            "xT": xTr[b],
            "wq": wqk_arr(Wq, hg), "wk": wqk_arr(Wk, hg), "wv": wv_arr(hg),
            "wo": wo_arr(hg),
            "cosF": cosF, "sinS": sinS, "tri": tri,
        })
    return in_maps


def prepare(**inputs):
    """Returns (nc, in_maps). Exposed for test.py's traced runs."""
    global _STATE
    if _STATE is None:
        _STATE = _build()
    return _STATE, _host_prep(**inputs)


def kernel(**inputs):
    nc, in_maps = prepare(**inputs)
    res = bass_utils.run_bass_kernel_spmd(nc, in_maps, core_ids=list(range(8)))
    out = np.empty((B, S, D), np.float32)
    for b in range(B):
        out[b] = res.results[2 * b]["out"] + res.results[2 * b + 1]["out"]
    return out


# revision 20
# speedup vs baseline: 1.8239x; 1.1524x over previous
"""Causal multi-head self-attention (RoPE) Trainium2 Bass kernel.

Problem: x[4,2048,1024] f32, Wq/Wk/Wv/Wo[1024,1024], token_positions[2048].
  q,k,v = x@W.T per head (16 heads, dk=64); RoPE(q,k); causal softmax(q k^T/8) @ v;
  concat heads @ Wo.T.

Sharding (8 cores): core c -> batch b=c//2, head-group hg=c%2 (8 heads each).
Each core computes a partial output (its 8 heads' contribution through Wo);
host sums the two partials per batch.

On-chip layouts (per core):
  xT      [128, 8, 2048] f32r   : x[b].T chunked over d_model (DMA-streamed)
  qT/kT   [128, 512] bf16 per (pair, s-tile): rows = rope-permuted dims of a
          head pair: [A-even(0:32) A-odd(32:64) B-even(64:96) B-odd(96:128)]
  v       [128, 512] bf16 per s-chunk (8 heads x 64)
  scoresT [128, 2, 512] psum per chunk; exp on ACT -> attnT bf16; causal mask
          via gpsimd affine_select on diagonal blocks; AV pair col-tiled into
          one psum bank; denominator via ones[128,64] matmuls (broadcast over
          64 rows) into a second bank; one reciprocal_approx_fast + one
          tensor_mul normalizes; Wo projection from normalized outT.
"""

import os
from contextlib import ExitStack

import numpy as np
import ml_dtypes

import concourse.bass as bass
import concourse.tile as tile
from concourse import bacc, mybir
from concourse import bass_utils
from concourse._compat import with_exitstack

P = 128
B, S, D = 4, 2048, 1024
NHEAD, DK = 16, 64
HPC = 8      # heads per core
NPAIR = 4    # head pairs per core
DCH = 8      # d_model 128-chunks
NQT = 4      # q tiles of 512
SQT = 512
THETA = 10000.0
SCALE = 0.125          # 1/sqrt(dk)

F32 = mybir.dt.float32
F32R = mybir.dt.float32r
BF16 = mybir.dt.bfloat16

ROWSPLIT = os.environ.get("K_ROWSPLIT", "0") == "1"
PROJ16 = os.environ.get("K_PROJ16", "1") == "1"
PDT = BF16 if PROJ16 else F32R
PNP = ml_dtypes.bfloat16 if PROJ16 else np.float32
_STATE = None  # compile cache


@with_exitstack
def _attn_kernel(ctx: ExitStack, tc: tile.TileContext, out_ap, ins):
    nc = tc.nc
    xT, wq, wk, wv, wo, cosF, sinS, tri = ins

    wpool = ctx.enter_context(tc.tile_pool(name="w", bufs=1))
    xpool = ctx.enter_context(tc.tile_pool(name="x", bufs=2))
    qkpool = ctx.enter_context(tc.tile_pool(name="qk", bufs=1))
    vpool = ctx.enter_context(tc.tile_pool(name="v", bufs=1))
    rpool = ctx.enter_context(tc.tile_pool(name="rope", bufs=4))
    apool = ctx.enter_context(tc.tile_pool(name="attn", bufs=4))
    npool = ctx.enter_context(tc.tile_pool(name="norm", bufs=1))
    rcpool = ctx.enter_context(tc.tile_pool(name="rcp", bufs=2))
    wopool = ctx.enter_context(tc.tile_pool(name="wos", bufs=3))
    # PSUM (8 banks): psS 2x[128,2,512]=4, psO 1x{o0,o1}=2, psM 2x[128,512]=2
    psS = ctx.enter_context(tc.tile_pool(name="psS", bufs=2, space="PSUM"))
    psO = ctx.enter_context(tc.tile_pool(name="psO", bufs=1, space="PSUM"))
    psM = ctx.enter_context(tc.tile_pool(name="psM", bufs=2, space="PSUM"))

    # ---- resident constants ----
    wq_sb = wpool.tile([P, DCH, NPAIR, P], PDT, tag="wq")
    nc.sync.dma_start(wq_sb[:], wq)
    wk_sb = wpool.tile([P, DCH, NPAIR, P], PDT, tag="wk")
    nc.sync.dma_start(wk_sb[:], wk)
    wv_sb = wpool.tile([P, DCH, HPC * DK], PDT, tag="wv")
    nc.sync.dma_start(wv_sb[:], wv)
    wo_sb = wpool.tile([P, NPAIR, D], BF16, tag="wo")
    nc.sync.dma_start(wo_sb[:], wo)
    cos_sb = wpool.tile([P, S], BF16, tag="cos")
    nc.sync.dma_start(cos_sb[:], cosF)
    sin_sb = wpool.tile([P, S], BF16, tag="sin")
    nc.sync.dma_start(sin_sb[:], sinS)
    tri_sb = wpool.tile([P, P], BF16, tag="tri")
    nc.sync.dma_start(tri_sb[:], tri)

    qk_tiles = {}   # (proj, pair, stile) -> tile [128, 512] bf16
    v_tiles = {}    # schunk -> tile [128, 512] bf16
    nrm_tiles = {}  # (pair, qtile) -> tile [128, 512] bf16

    exp_fn = mybir.ActivationFunctionType.Exp

    def phase_a(t):
        xb = xpool.tile([P, DCH, SQT], PDT, tag="xb")
        nc.sync.dma_start(xb[:], xT[:, :, t * SQT:(t + 1) * SQT])
        s_sl = slice(t * SQT, (t + 1) * SQT)
        for p in range(NPAIR):
            for proj, w_sb in (("q", wq_sb), ("k", wk_sb)):
                ps = psM.tile([P, SQT], F32, tag="m")
                if ROWSPLIT:
                    for c in range(DCH):
                        for hf in range(2):
                            nc.tensor.matmul(
                                ps[:], w_sb[64 * hf:64 * hf + 64, c, p],
                                xb[64 * hf:64 * hf + 64, c],
                                start=(c == 0 and hf == 0),
                                stop=(c == DCH - 1 and hf == 1))
                else:
                    for c in range(DCH):
                        nc.tensor.matmul(ps[:], w_sb[:, c, p], xb[:, c],
                                         start=(c == 0), stop=(c == DCH - 1))
                # RoPE: one psum evacuation (releases the psum bank fast),
                # swap-copies on idle GPSIMD, bf16 2x multiplies/add on DVE.
                pb = rpool.tile([P, SQT], BF16, tag="pb")
                nc.vector.tensor_copy(pb[:], ps[:])
                sw = rpool.tile([P, SQT], BF16, tag="sw")
                for blk, src in ((0, 32), (1, 0), (2, 96), (3, 64)):
                    nc.sync.dma_start(sw[32 * blk:32 * blk + 32],
                                      pb[src:src + 32])
                u = rpool.tile([P, SQT], BF16, tag="u")
                nc.vector.tensor_mul(u[:], pb[:], cos_sb[:, s_sl])
                w_ = rpool.tile([P, SQT], BF16, tag="wt")
                nc.vector.tensor_mul(w_[:], sw[:], sin_sb[:, s_sl])
                qt = qkpool.tile([P, SQT], BF16, tag=f"{proj}{p}_{t % 2 if proj == chr(113) else t}")
                nc.vector.tensor_add(qt[:], u[:], w_[:])
                qk_tiles[(proj, p, t)] = qt
        for sc4 in range(4):
            sc = 4 * t + sc4
            ps = psM.tile([P, SQT], F32, tag="m")
            if ROWSPLIT:
                for c in range(DCH):
                    for hf in range(2):
                        nc.tensor.matmul(
                            ps[:], xb[64 * hf:64 * hf + 64, c, 128 * sc4:128 * sc4 + 128],
                            wv_sb[64 * hf:64 * hf + 64, c],
                            start=(c == 0 and hf == 0),
                            stop=(c == DCH - 1 and hf == 1))
            else:
                for c in range(DCH):
                    nc.tensor.matmul(ps[:], xb[:, c, 128 * sc4:128 * sc4 + 128],
                                     wv_sb[:, c], start=(c == 0), stop=(c == DCH - 1))
            va = vpool.tile([P, HPC, 2 * DK], BF16, tag=f"v{sc}")
            nc.vector.tensor_copy(
                va[:, :, 0:DK], ps[:].rearrange("p (h d) -> p h d", d=DK))
            nc.vector.memset(va[:, :, DK:2 * DK], 1.0)
            v_tiles[sc] = va

    def phase_b(t):
        for p in range(NPAIR):
            qt = qk_tiles[("q", p, t)]
            oh = [psO.tile([P, SQT], F32, tag=f"o{h}", name=f"oh{h}")
                  for h in range(2)]
            nch = 4 * t + 4
            for kc in range(nch):
                delta = max(0, 128 * kc - SQT * t)
                kt = qk_tiles[("k", p, kc // 4)]
                ci = kc % 4
                sT = psS.tile([P, 2, SQT], F32, tag="s")
                for h in range(2):
                    nc.tensor.matmul(
                        sT[:, h, delta:], kt[64 * h:64 * h + 64, 128 * ci:128 * ci + 128],
                        qt[64 * h:64 * h + 64, delta:], start=True, stop=True)
                at = apool.tile([P, 2, SQT], BF16, tag="a")
                nc.scalar.activation(at[:, :, delta:], sT[:, :, delta:], exp_fn,
                                     scale=SCALE)
                if 128 * kc >= SQT * t:
                    # diagonal block: zero attn where q < k (gpsimd)
                    for h in range(2):
                        nc.gpsimd.affine_select(
                            out=at[:, h, delta:delta + 128],
                            in_=at[:, h, delta:delta + 128],
                            compare_op=mybir.AluOpType.is_ge,
                            fill=0.0, base=0,
                            pattern=[[1, 128]], channel_multiplier=-1)
                va = v_tiles[kc]
                st_, sp_ = (kc == 0), (kc == nch - 1)
                for h in range(2):
                    nc.tensor.matmul(
                        oh[h][:, delta:], va[:, 2 * p + h, :],
                        at[:, h, delta:], start=st_, stop=sp_)
            onrm = npool.tile([P, SQT], BF16, tag=f"n{p}_{t % 2}")
            ohb = rcpool.tile([P, SQT], F32, tag="ohb")
            dnb = rcpool.tile([P, SQT], F32, tag="dnb")
            for h in range(2):
                # evacuate outT+denom; psum slot released after these copies
                nc.vector.tensor_copy(ohb[64 * h:64 * h + 64], oh[h][0:64, :])
                nc.vector.tensor_copy(dnb[64 * h:64 * h + 64], oh[h][64:128, :])
            rc = rcpool.tile([P, SQT], F32, tag="rc")
            nc.vector.reciprocal_approx_fast(rc[:], dnb[:])
            nc.vector.tensor_mul(onrm[:], ohb[:], rc[:])
            nrm_tiles[(p, t)] = onrm

    def phase_wo(t):
        for qs in range(4):
            for nh in range(2):
                wps = psO.tile([P, SQT], F32, tag=f"o{(2 * qs + nh) % 2}",
                               name="wps")
                for p in range(NPAIR):
                    if ROWSPLIT:
                        for hf in range(2):
                            nc.tensor.matmul(
                                wps[:],
                                nrm_tiles[(p, t)][64 * hf:64 * hf + 64,
                                                  128 * qs:128 * qs + 128],
                                wo_sb[64 * hf:64 * hf + 64, p,
                                      SQT * nh:SQT * (nh + 1)],
                                start=(p == 0 and hf == 0),
                                stop=(p == NPAIR - 1 and hf == 1))
                    else:
                        nc.tensor.matmul(
                            wps[:], nrm_tiles[(p, t)][:, 128 * qs:128 * qs + 128],
                            wo_sb[:, p, SQT * nh:SQT * (nh + 1)],
                            start=(p == 0), stop=(p == NPAIR - 1))
                st = wopool.tile([P, SQT], F32, tag="wo")
                if nh == 0:
                    nc.vector.tensor_copy(st[:], wps[:])
                else:
                    nc.scalar.copy(st[:], wps[:])
                nc.sync.dma_start(
                    out_ap[SQT * t + 128 * qs:SQT * t + 128 * qs + 128,
                           SQT * nh:SQT * (nh + 1)], st[:])

    for t in range(NQT):
        phase_a(t)
        phase_b(t)
        phase_wo(t)


def _build():
    nc = bacc.Bacc("TRN2", target_bir_lowering=False, debug=False, num_devices=8)
    ins = [
        nc.dram_tensor("xT", [P, DCH, S], PDT, kind="ExternalInput").ap(),
        nc.dram_tensor("wq", [P, DCH, NPAIR, P], PDT, kind="ExternalInput").ap(),
        nc.dram_tensor("wk", [P, DCH, NPAIR, P], PDT, kind="ExternalInput").ap(),
        nc.dram_tensor("wv", [P, DCH, HPC * DK], PDT, kind="ExternalInput").ap(),
        nc.dram_tensor("wo", [P, NPAIR, D], BF16, kind="ExternalInput").ap(),
        nc.dram_tensor("cosF", [P, S], BF16, kind="ExternalInput").ap(),
        nc.dram_tensor("sinS", [P, S], BF16, kind="ExternalInput").ap(),
        nc.dram_tensor("tri", [P, P], BF16, kind="ExternalInput").ap(),
    ]
    out_ap = nc.dram_tensor("out", [S, D], F32, kind="ExternalOutput").ap()
    with tile.TileContext(nc) as tc:
        _attn_kernel(tc, out_ap, ins)
    nc.compile()
    return nc


def _host_prep(x, Wq, Wk, Wv, Wo, token_positions):
    """Build the 8 per-core input maps."""
    x = np.asarray(x, dtype=np.float32)
    Wq = np.asarray(Wq, dtype=np.float32)
    Wk = np.asarray(Wk, dtype=np.float32)
    Wv = np.asarray(Wv, dtype=np.float32)
    Wo = np.asarray(Wo, dtype=np.float32)
    pos = np.asarray(token_positions).astype(np.float64)

    # RoPE tables: rows 0:32 freq-major (even dims), repeated for the 4
    # 32-row blocks; sin signed [-,+,-,+] to implement the rotation swap.
    freqs = 1.0 / (THETA ** (np.arange(0, DK, 2, dtype=np.float64) / DK))  # [32]
    ang = pos[:, None] * freqs[None, :]          # [S, 32]
    cosT = np.cos(ang).T.astype(np.float32)      # [32, S]
    sinT = np.sin(ang).T.astype(np.float32)
    cosF = np.tile(cosT, (4, 1)).astype(ml_dtypes.bfloat16)
    sinS = np.concatenate([-sinT, sinT, -sinT, sinT], 0).astype(ml_dtypes.bfloat16)
    kk = np.arange(P)[:, None]
    qq = np.arange(P)[None, :]
    tri = np.where(qq >= kk, 0.0, -30000.0).astype(ml_dtypes.bfloat16)

    xTr = [np.ascontiguousarray(
        x[b].T.reshape(DCH, P, S).transpose(1, 0, 2)).astype(PNP) for b in range(B)]

    def wqk_arr(W, hg):
        perm = np.empty((NPAIR, P), np.int64)
        for p in range(NPAIR):
            hA, hB = 8 * hg + 2 * p, 8 * hg + 2 * p + 1
            perm[p] = np.concatenate([
                DK * hA + np.arange(0, DK, 2), DK * hA + np.arange(1, DK, 2),
                DK * hB + np.arange(0, DK, 2), DK * hB + np.arange(1, DK, 2)])
        a = W[perm]                                  # [4, 128, 1024]
        a = a.reshape(NPAIR, P, DCH, P).transpose(3, 2, 0, 1)  # [pi, c, p, m]
        return np.ascontiguousarray(a).astype(PNP)

    def wv_arr(hg):
        a = Wv[DK * HPC * hg: DK * HPC * (hg + 1)].T   # [1024, 512]
        return np.ascontiguousarray(
            a.reshape(DCH, P, HPC * DK).transpose(1, 0, 2)).astype(PNP)

    def wo_arr(hg):
        a = Wo[:, DK * HPC * hg: DK * HPC * (hg + 1)].T  # [512, 1024]
        return np.ascontiguousarray(
            a.reshape(NPAIR, P, D).transpose(1, 0, 2)).astype(ml_dtypes.bfloat16)

    in_maps = []
    for c in range(8):
        b, hg = c // 2, c % 2
        in_maps.append({
            "xT": xTr[b],
            "wq": wqk_arr(Wq, hg), "wk": wqk_arr(Wk, hg), "wv": wv_arr(hg),
            "wo": wo_arr(hg),
            "cosF": cosF, "sinS": sinS, "tri": tri,
        })
    return in_maps


def prepare(**inputs):
    """Returns (nc, in_maps). Exposed for test.py's traced runs."""
    global _STATE
    if _STATE is None:
        _STATE = _build()
    return _STATE, _host_prep(**inputs)


def kernel(**inputs):
    nc, in_maps = prepare(**inputs)
    res = bass_utils.run_bass_kernel_spmd(nc, in_maps, core_ids=list(range(8)))
    out = np.empty((B, S, D), np.float32)
    for b in range(B):
        out[b] = res.results[2 * b]["out"] + res.results[2 * b + 1]["out"]
    return out
